# revision 33
# baseline (speedup 1.0000x reference)
"""CenterLoss kernel for Trainium2 (8 NeuronCores, data-parallel over batch).

loss = mean_i( clip( ||x_i - centers[labels[i]]||^2, 1e-12, 1e12 ) )

Gather the labeled center row per sample with indirect DMA and compute the
squared distance directly: O(B*D) work instead of O(B*C*D).

Sharding: x/labels split into 8 batch shards of 1024 rows; centers replicated.
Host sums the 8 partial outputs and divides by global B.

Perf notes (final, ~27.3us vs the 338us session baseline):
  - The SWDGE ring holds only 128 in-flight descriptors per queue, so the
    gather is 8 indirect calls x 128 rows; merged calls overflow the ring
    and serialize at ~320ns/descriptor. Calls alternate between two SWDGE
    queues (num_swdge_queues=2, queue patched on the emitted InstDMACopy).
    Each call costs ~1.4us of GPSIMD ucode (994ns fixed) — the dominant
    serial chain of the kernel.
  - x and centers ship as bf16 (host-converted): halves every DMA byte and
    doubles DVE throughput. Per-sample accumulation stays fp32 (ACT
    accumulator / DVE accumulator): ~1e-5 relative error on the mean vs
    the fp32 reference, far inside the 2e-2 gate.
  - The idx load is issued BEFORE TileContext entry (manual semaphore,
    wait attached to the first gather after tile scheduling), skipping the
    tile entry drains so descriptor-gen starts ~1us earlier.
  - DVE row-sums use one fused scalar_tensor_tensor (square via
    (diff bypass) * diff with free-axis accumulation) instead of
    mult + tensor_reduce; its accumulator read is 81ns vs ACT's 278ns.

Per-core layout (B_loc=1024, P=128 partitions, M=8 row-chunks):
  sample s lives at (partition p, chunk m) with s = p*8 + m; the x DMA
  reads 8KB contiguous bf16 per partition. idx[p, m] = labels[p*8+m] int32;
  gather call m uses offset AP idx[:, m] (per-partition column — a
  single-partition-row offset AP crashes the HW SWDGE).
"""

import sys

import numpy as np

if "/opt/trn_rl_repo" not in sys.path:
    sys.path.insert(0, "/opt/trn_rl_repo")

import ml_dtypes

_B, _D, _C = 8192, 512, 8000
_N_CORES = 8
_B_LOC = _B // _N_CORES  # 1024 rows per core
_P = 128
_M = _B_LOC // _P  # 8 chunks of 128 rows
_N_QUEUES = 2
_CLAMP_MIN, _CLAMP_MAX = 1e-12, 1e12

_cache: dict = {}


def _build():
    import concourse.bass as bass
    import concourse.tile as tile
    from concourse import bacc, mybir

    nc = bacc.Bacc(
        "TRN2",
        debug=False,
        enable_asserts=False,
        target_bir_lowering=False,
        num_devices=_N_CORES,
        num_swdge_queues=_N_QUEUES,
        use_seq_codegen=True,
    )
    x_d = nc.dram_tensor("x", [_B_LOC, _D], mybir.dt.bfloat16, kind="ExternalInput")
    # labels arrive host-packed as idx[p, m] = labels[p*8 + m] (see kernel()).
    lab_d = nc.dram_tensor("labels_packed", [_P, _M], mybir.dt.int32, kind="ExternalInput")
    cen_d = nc.dram_tensor("centers", [_C, _D], mybir.dt.bfloat16, kind="ExternalInput")
    out_d = nc.dram_tensor("out", [_P, _M], mybir.dt.float32, kind="ExternalOutput")

    # Issue the idx load BEFORE TileContext entry: it skips the tile
    # framework's entry drains/ordering, shaving ~1us off the critical
    # path (all gather descriptor-gen waits on this DMA). Hand-synced via
    # idx_sem -> gpsimd.wait_ge before the first indirect call.
    idx_sb = nc.alloc_sbuf_tensor("idx_early", [_P, _M], mybir.dt.int32)
    idx_sem = nc.alloc_semaphore("idx_sem")
    nc.sync.dma_start(out=idx_sb[:], in_=lab_d.ap()).then_inc(idx_sem, 16)

    with tile.TileContext(nc) as tc:
        with (
            tc.tile_pool(name="big", bufs=1) as big,
            tc.tile_pool(name="work", bufs=4) as work,
            tc.tile_pool(name="misc", bufs=1) as misc,
        ):
            idx = idx_sb
            gather_h = []

            xsb = big.tile([_P, _M * _D], mybir.dt.bfloat16)
            nc.sync.dma_start(
                out=xsb[:], in_=x_d.ap().rearrange("(p m) d -> p (m d)", p=_P)
            )

            dist = misc.tile([_P, _M], mybir.dt.float32)

            g = big.tile([_P, _M * _D], mybir.dt.bfloat16)
            g3 = g[:].rearrange("p (m d) -> p m d", d=_D)
            _DVE_SQ = {4, 6}  # chunks whose square+rowsum runs on DVE
            for m in range(_M):
                h = nc.gpsimd.indirect_dma_start(
                    out=g3[:, m, :],
                    out_offset=None,
                    in_=cen_d.ap(),
                    in_offset=bass.IndirectOffsetOnAxis(
                        ap=idx[:, m : m + 1], axis=0
                    ),
                )
                # Alternate SWDGE queues so ring await_space never stalls
                # the next call's descriptor generation.
                if m % _N_QUEUES:
                    h.ins.queue = "qPoolDynamic1"
                gather_h.append(h)
                diff = work.tile([_P, _D], mybir.dt.bfloat16, tag="diff")
                nc.vector.tensor_tensor(
                    out=diff[:],
                    in0=xsb[:, m * _D : (m + 1) * _D],
                    in1=g[:, m * _D : (m + 1) * _D],
                    op=mybir.AluOpType.subtract,
                )
                if m not in _DVE_SQ:
                    # fused square + fp32 row-sum on the scalar engine
                    sq = work.tile([_P, _D], mybir.dt.bfloat16, tag="sq")
                    nc.scalar.activation(
                        out=sq[:],
                        in_=diff[:],
                        func=mybir.ActivationFunctionType.Square,
                        accum_out=dist[:, m : m + 1],
                    )
                else:
                    # balance engines: one fused DVE op — square via
                    # (diff bypass) * diff with free-axis accumulation
                    sq = work.tile([_P, _D], mybir.dt.bfloat16, tag="sqv")
                    nc.vector.scalar_tensor_tensor(
                        out=sq[:],
                        in0=diff[:],
                        scalar=0.0,
                        in1=diff[:],
                        op0=mybir.AluOpType.bypass,
                        op1=mybir.AluOpType.mult,
                        accum_out=dist[:, m : m + 1],
                    )

            # clip both bounds in one DVE op: out = min(max(dist, lo), hi).
            # Columns 0-6 clip as soon as chunk 6 lands; only column 7's tiny
            # clip trails the final accum, so the out-DMA fires sooner.
            nc.vector.tensor_scalar(
                out=dist[:, : _M - 1],
                in0=dist[:, : _M - 1],
                scalar1=_CLAMP_MIN,
                scalar2=_CLAMP_MAX,
                op0=mybir.AluOpType.max,
                op1=mybir.AluOpType.min,
            )
            nc.vector.tensor_scalar(
                out=dist[:, _M - 1 :],
                in0=dist[:, _M - 1 :],
                scalar1=_CLAMP_MIN,
                scalar2=_CLAMP_MAX,
                op0=mybir.AluOpType.max,
                op1=mybir.AluOpType.min,
            )

            nc.sync.dma_start(out=out_d.ap()[:, :], in_=dist[:])
    # Attach the idx-DMA wait to the first gather AFTER tile scheduling
    # (the scheduler's block-local sim cannot see the pre-tile DMA's
    # increment and would deadlock on an in-block wait).
    gather_h[0].wait_op(idx_sem, 16, "sem-ge")
    nc.compile()
    return nc


def _pack_labels(labels_shard: np.ndarray) -> np.ndarray:
    """idx[p, m] = labels[p*8 + m], int32, shape [128, 8]."""
    return np.ascontiguousarray(labels_shard.reshape(_P, _M).astype(np.int32))


def _run(x, labels, centers, trace=False, **hw_kwargs):
    from concourse import bass_utils

    if "nc" not in _cache:
        _cache["nc"] = _build()
    nc = _cache["nc"]

    x = np.asarray(x, dtype=np.float32).astype(ml_dtypes.bfloat16)
    labels = np.ascontiguousarray(np.asarray(labels).astype(np.int64))
    centers = np.ascontiguousarray(
        np.asarray(centers, dtype=np.float32).astype(ml_dtypes.bfloat16)
    )
    assert x.shape == (_B, _D) and labels.shape == (_B,) and centers.shape == (_C, _D)
    assert labels.min() >= 0 and labels.max() < _C

    in_maps = []
    for c in range(_N_CORES):
        sl = slice(c * _B_LOC, (c + 1) * _B_LOC)
        in_maps.append(
            {
                "x": np.ascontiguousarray(x[sl]),
                "labels_packed": _pack_labels(labels[sl]),
                "centers": centers,
            }
        )

    r = bass_utils.run_bass_kernel_spmd(
        nc, in_maps, core_ids=list(range(_N_CORES)), trace=trace, **hw_kwargs
    )
    total = sum(res["out"].astype(np.float64).sum() for res in r.results)
    return np.array(total / _B, dtype=np.float32), r


def kernel(x, labels, centers):
    out, _ = _run(x, labels, centers, trace=False)
    return out
